# revision 9
# baseline (speedup 1.0000x reference)
"""Causal single-head attention on 8 trn2 cores, data-parallel over batch.

Reference: q,k,v = x@Wq, x@Wk, x@Wv;  wei = softmax(mask(q k^T * C^-0.5));
out = wei @ v.   B,T,C,HS = 512,256,256,64.

Strategy per core (64 batch items):
 - host pre-transposes x -> xT [B, C, T] and folds C^-0.5 into Wq.
 - qkT [128,256] = [Wq'|Wk]^T @ xT_b  (rows 0:64 = qT, 64:128 = kT)
 - v [256,64] natural layout via lhsT=xT chunks.
 - weiT [tk, tq] blocks via lhsT=kT-chunk, rhs=qT (fp32r, N=256 -> full rate)
 - additive causal mask (-1e5) + exp on ScalarE -> expT in SBUF
 - out_aug [65, 256] = [v|1]^T @ expT  (row 64 = softmax denominator)
 - host: out = (out_aug[:64]/out_aug[64]).T
"""

import os
import numpy as np

B, T, C, HS = 512, 256, 256, 64
NCORES = 8
BPC = B // NCORES  # 64 items per core

LAST_RESULT = None  # stash for test harness (exec_time_ns, trace dirs)

_COMPILED = None  # cache (nc, names) across calls


def _build_program():
    import concourse.bass as bass
    import concourse.mybir as mybir
    import concourse.bacc as bacc
    import concourse.tile as tile

    f32 = mybir.dt.float32
    f32r = mybir.dt.float32r
    AF = mybir.ActivationFunctionType

    nc = bacc.Bacc(
        "TRN2",
        target_bir_lowering=False,
        debug=False,
        enable_asserts=True,
        num_devices=NCORES,
    )

    xT_d = nc.dram_tensor("xT", [BPC, C, T], f32r, kind="ExternalInput").ap()
    wqk_d = nc.dram_tensor("Wqk", [C, 128], f32r, kind="ExternalInput").ap()  # [c, q|k]
    wv_d = nc.dram_tensor("Wv", [C, HS], f32r, kind="ExternalInput").ap()
    mask_d = nc.dram_tensor("maskT", [128, 512], f32, kind="ExternalInput").ap()
    ones_d = nc.dram_tensor("ones", [128, 2], f32r, kind="ExternalInput").ap()
    out_d = nc.dram_tensor("out_aug", [BPC, HS + 1, T], f32, kind="ExternalOutput").ap()

    with tile.TileContext(nc) as tc:
        from contextlib import ExitStack

        with ExitStack() as ctx:
            const = ctx.enter_context(tc.tile_pool(name="const", bufs=1))
            xpool = ctx.enter_context(tc.tile_pool(name="x", bufs=3))
            qkp = ctx.enter_context(tc.tile_pool(name="qkp", bufs=2, space="PSUM"))
            qks = ctx.enter_context(tc.tile_pool(name="qks", bufs=2))
            vp = ctx.enter_context(tc.tile_pool(name="vp", bufs=2, space="PSUM"))
            vs = ctx.enter_context(tc.tile_pool(name="vs", bufs=2))
            weip = ctx.enter_context(tc.tile_pool(name="weip", bufs=2, space="PSUM"))
            exps = ctx.enter_context(tc.tile_pool(name="exps", bufs=2))
            outp = ctx.enter_context(tc.tile_pool(name="outp", bufs=2, space="PSUM"))
            outs = ctx.enter_context(tc.tile_pool(name="outs", bufs=3))

            # constants
            wqk_sb = const.tile([128, 256], f32r)  # [c-chunk part, 2*128 M cols]
            nc.sync.dma_start(wqk_sb[:, 0:128], wqk_d[0:128, :])
            nc.sync.dma_start(wqk_sb[:, 128:256], wqk_d[128:256, :])
            wv_sb = const.tile([128, 128], f32r)  # cols 0:64 c-chunk0, 64:128 c-chunk1
            nc.sync.dma_start(wv_sb[:, 0:64], wv_d[0:128, :])
            nc.sync.dma_start(wv_sb[:, 64:128], wv_d[128:256, :])
            mask_sb = const.tile([128, 512], f32)
            nc.sync.dma_start(mask_sb[:], mask_d[:])
            ones_sb = const.tile([128, 2], f32r)
            nc.sync.dma_start(ones_sb[:], ones_d[:])

            r = lambda ap: ap  # tiles already f32r

            for b in range(BPC):
                # x^T for this item: cols 0:256 = c-chunk0 (t 0:256), 256:512 = c-chunk1
                xt = xpool.tile([128, 512], f32r)
                nc.sync.dma_start(xt[:, 0:256], xT_d[b, 0:128, :])
                nc.sync.dma_start(xt[:, 256:512], xT_d[b, 128:256, :])

                # qT | kT side by side: [64, 512] (cols 0:256 qT*scale, 256:512 kT)
                qk_ps = qkp.tile([64, 512], f32)
                nc.tensor.matmul(qk_ps[:, 0:256], r(wqk_sb[:, 0:64]), r(xt[:, 0:256]),
                                 start=True, stop=False)
                nc.tensor.matmul(qk_ps[:, 0:256], r(wqk_sb[:, 128:192]), r(xt[:, 256:512]),
                                 start=False, stop=True)
                nc.tensor.matmul(qk_ps[:, 256:512], r(wqk_sb[:, 64:128]), r(xt[:, 0:256]),
                                 start=True, stop=False)
                nc.tensor.matmul(qk_ps[:, 256:512], r(wqk_sb[:, 192:256]), r(xt[:, 256:512]),
                                 start=False, stop=True)
                qk_sb = qks.tile([64, 512], f32r)
                if b % 2 == 0:
                    nc.scalar.copy(qk_sb[:], qk_ps[:])
                else:
                    nc.vector.tensor_copy(qk_sb[:], qk_ps[:])

                # v natural [t, h]: psum [128, 128]: cols 0:64 t-chunk0, 64:128 t-chunk1
                v_ps = vp.tile([128, 128], f32)
                nc.tensor.matmul(v_ps[:, 0:64], r(xt[:, 0:128]), r(wv_sb[:, 0:64]),
                                 start=True, stop=False)
                nc.tensor.matmul(v_ps[:, 0:64], r(xt[:, 256:384]), r(wv_sb[:, 64:128]),
                                 start=False, stop=True)
                nc.tensor.matmul(v_ps[:, 64:128], r(xt[:, 128:256]), r(wv_sb[:, 0:64]),
                                 start=True, stop=False)
                nc.tensor.matmul(v_ps[:, 64:128], r(xt[:, 384:512]), r(wv_sb[:, 64:128]),
                                 start=False, stop=True)
                # v_aug [tk-chunk part, 2*(64+1)]: cols 0:64 v(t0), 64 ones, 65:129 v(t1), 129 ones
                v_sb = vs.tile([128, 130], f32r)
                nc.vector.tensor_copy(
                    v_sb[:].rearrange("p (g d) -> p g d", d=65)[:, :, 0:64],
                    v_ps[:].rearrange("p (g d) -> p g d", d=64),
                )
                nc.vector.tensor_copy(
                    v_sb[:].rearrange("p (g d) -> p g d", d=65)[:, :, 64:65],
                    ones_sb[:].rearrange("p (g d) -> p g d", d=1),
                )

                # scoresT: [tk-chunk, tq 0:256] for tk-chunk 0 and 1 side by side
                wei_ps = weip.tile([128, 512], f32)
                nc.tensor.matmul(wei_ps[:, 0:256], r(qk_sb[:, 256:384]),
                                 r(qk_sb[:, 0:256]), start=True, stop=True)
                nc.tensor.matmul(wei_ps[:, 256:512], r(qk_sb[:, 384:512]),
                                 r(qk_sb[:, 0:256]), start=True, stop=True)
                # causal mask (additive -1e5) then exp -> SBUF
                nc.vector.tensor_add(wei_ps[:], wei_ps[:], mask_sb[:])
                expT_sb = exps.tile([128, 512], f32r)
                nc.scalar.activation(expT_sb[:], wei_ps[:], AF.Exp)

                # out_aug^T [65, 256]: rows 0:64 = (expT^T v)^T, row 64 = denom
                out_ps = outp.tile([HS + 1, 256], f32)
                nc.tensor.matmul(out_ps[:], r(v_sb[:, 0:65]), r(expT_sb[:, 0:256]),
                                 start=True, stop=False)
                nc.tensor.matmul(out_ps[:], r(v_sb[:, 65:130]), r(expT_sb[:, 256:512]),
                                 start=False, stop=True)
                out_sb = outs.tile([HS + 1, 256], f32)
                if b % 2 == 0:
                    nc.vector.tensor_copy(out_sb[:], out_ps[:])
                else:
                    nc.scalar.copy(out_sb[:], out_ps[:])
                nc.sync.dma_start(out_d[b], out_sb[:])

    nc.compile()
    return nc


def kernel(x, Wq, Wk, Wv):
    global LAST_RESULT, _COMPILED
    from concourse import bass_utils

    if _COMPILED is None:
        _COMPILED = _build_program()
    nc = _COMPILED

    scale = np.float32(C ** -0.5)
    wqk = np.concatenate([Wq.astype(np.float32) * scale, Wk.astype(np.float32)],
                         axis=1)  # [C, 128]
    xT = np.ascontiguousarray(x.astype(np.float32).transpose(0, 2, 1))  # [B, C, T]

    # additive causal mask in weiT layout [tk(2 chunks of 128), tq 256]
    tq = np.arange(T)
    mask = np.zeros((128, 512), dtype=np.float32)
    for chunk in range(2):
        tk = np.arange(128) + chunk * 128
        blk = np.where(tq[None, :] >= tk[:, None], 0.0, -1e5).astype(np.float32)
        mask[:, chunk * 256:(chunk + 1) * 256] = blk

    in_maps = []
    for ci in range(NCORES):
        in_maps.append({
            "xT": xT[ci * BPC:(ci + 1) * BPC],
            "Wqk": wqk,
            "Wv": np.ascontiguousarray(Wv.astype(np.float32)),
            "maskT": mask,
            "ones": np.ones((128, 2), dtype=np.float32),
        })

    res = bass_utils.run_bass_kernel_spmd(nc, in_maps, core_ids=list(range(NCORES)))
    LAST_RESULT = res

    out_aug = np.concatenate([r["out_aug"] for r in res.results], axis=0)  # [B,65,T]
    out = out_aug[:, :HS, :] / out_aug[:, HS:HS + 1, :]
    return np.ascontiguousarray(out.transpose(0, 2, 1)).astype(np.float32)


# revision 15
# speedup vs baseline: 3.1848x; 3.1848x over previous
"""Causal single-head attention on 8 trn2 cores, data-parallel over batch.

Reference: q,k,v = x@Wq, x@Wk, x@Wv;  wei = softmax(mask(q k^T * C^-0.5));
out = wei @ v.   B,T,C,HS = 512,256,256,64.

Strategy per core (64 batch items):
 - host pre-transposes x -> xT [B, C, T] and folds C^-0.5 into Wq.
 - qkT [128,256] = [Wq'|Wk]^T @ xT_b  (rows 0:64 = qT, 64:128 = kT)
 - v [256,64] natural layout via lhsT=xT chunks.
 - weiT [tk, tq] blocks via lhsT=kT-chunk, rhs=qT (fp32r, N=256 -> full rate)
 - additive causal mask (-1e5) + exp on ScalarE -> expT in SBUF
 - out_aug [65, 256] = [v|1]^T @ expT  (row 64 = softmax denominator)
 - host: out = (out_aug[:64]/out_aug[64]).T
"""

import os
import numpy as np

B, T, C, HS = 512, 256, 256, 64
NCORES = 8
BPC = B // NCORES  # 64 items per core

LAST_RESULT = None  # stash for test harness (exec_time_ns, trace dirs)

_COMPILED = None  # cache (nc, names) across calls


def _build_program():
    import concourse.bass as bass
    import concourse.mybir as mybir
    import concourse.bacc as bacc
    import concourse.tile as tile

    f32 = mybir.dt.float32
    f32r = mybir.dt.float32r
    AF = mybir.ActivationFunctionType

    nc = bacc.Bacc(
        "TRN2",
        target_bir_lowering=False,
        debug=False,
        enable_asserts=True,
        num_devices=NCORES,
    )

    xT_d = nc.dram_tensor("xT", [BPC, C, T], f32r, kind="ExternalInput").ap()
    wqk_d = nc.dram_tensor("Wqk", [C, 128], f32r, kind="ExternalInput").ap()  # [c, q|k]
    wv_d = nc.dram_tensor("Wv", [C, HS], f32r, kind="ExternalInput").ap()
    mask_d = nc.dram_tensor("maskT", [128, 512], f32, kind="ExternalInput").ap()
    ones_d = nc.dram_tensor("ones", [128, 2], f32r, kind="ExternalInput").ap()
    maskm_d = nc.dram_tensor("maskM", [128, 512], f32r, kind="ExternalInput").ap()
    out_d = nc.dram_tensor("out_aug", [BPC, HS + 1, T], f32, kind="ExternalOutput").ap()

    with tile.TileContext(nc) as tc:
        from contextlib import ExitStack

        with ExitStack() as ctx:
            const = ctx.enter_context(tc.tile_pool(name="const", bufs=1))
            xpool = ctx.enter_context(tc.tile_pool(name="x", bufs=4))
            qkp = ctx.enter_context(tc.tile_pool(name="qkp", bufs=2, space="PSUM"))
            qks = ctx.enter_context(tc.tile_pool(name="qks", bufs=3))
            vp = ctx.enter_context(tc.tile_pool(name="vp", bufs=2, space="PSUM"))
            vs = ctx.enter_context(tc.tile_pool(name="vs", bufs=2))
            weip = ctx.enter_context(tc.tile_pool(name="weip", bufs=2, space="PSUM"))
            exps = ctx.enter_context(tc.tile_pool(name="exps", bufs=3))
            outp = ctx.enter_context(tc.tile_pool(name="outp", bufs=2, space="PSUM"))
            outs = ctx.enter_context(tc.tile_pool(name="outs", bufs=3))

            # constants
            wqk_sb = const.tile([128, 256], f32r)  # [c-chunk part, 2*128 M cols]
            nc.sync.dma_start(wqk_sb[:, 0:128], wqk_d[0:128, :])
            nc.sync.dma_start(wqk_sb[:, 128:256], wqk_d[128:256, :])
            wv_sb = const.tile([128, 128], f32r)  # cols 0:64 c-chunk0, 64:128 c-chunk1
            nc.sync.dma_start(wv_sb[:, 0:64], wv_d[0:128, :])
            nc.sync.dma_start(wv_sb[:, 64:128], wv_d[128:256, :])
            mask_sb = const.tile([128, 512], f32)
            nc.sync.dma_start(mask_sb[:], mask_d[:])
            ones_sb = const.tile([128, 2], f32r)
            nc.sync.dma_start(ones_sb[:], ones_d[:])
            mmask_sb = const.tile([128, 512], f32r)
            nc.sync.dma_start(mmask_sb[:], maskm_d[:])

            r = lambda ap: ap  # tiles already f32r

            for b in range(BPC):
                # x^T for this item: cols 0:256 = c-chunk0 (t 0:256), 256:512 = c-chunk1
                xt = xpool.tile([128, 512], f32r)
                nc.sync.dma_start(xt[:, 0:256], xT_d[b, 0:128, :])
                nc.sync.dma_start(xt[:, 256:512], xT_d[b, 128:256, :])

                # qT | kT side by side: [64, 512] (cols 0:256 qT*scale, 256:512 kT)
                qk_ps = qkp.tile([64, 512], f32)
                nc.tensor.matmul(qk_ps[:, 0:256], r(wqk_sb[:, 0:64]), r(xt[:, 0:256]),
                                 start=True, stop=False)
                nc.tensor.matmul(qk_ps[:, 0:256], r(wqk_sb[:, 128:192]), r(xt[:, 256:512]),
                                 start=False, stop=True)
                nc.tensor.matmul(qk_ps[:, 256:512], r(wqk_sb[:, 64:128]), r(xt[:, 0:256]),
                                 start=True, stop=False)
                nc.tensor.matmul(qk_ps[:, 256:512], r(wqk_sb[:, 192:256]), r(xt[:, 256:512]),
                                 start=False, stop=True)
                qk_sb = qks.tile([64, 512], f32r)
                if b % 2 == 0:
                    nc.scalar.copy(qk_sb[:], qk_ps[:])
                else:
                    nc.vector.tensor_copy(qk_sb[:], qk_ps[:])

                # scoresT: [tk-chunk, tq 0:256] for tk-chunk 0 and 1 side by side
                wei_ps = weip.tile([128, 512], f32)
                nc.tensor.matmul(wei_ps[:, 0:256], r(qk_sb[:, 256:384]),
                                 r(qk_sb[:, 0:256]), start=True, stop=True)
                nc.tensor.matmul(wei_ps[:, 256:512], r(qk_sb[:, 384:512]),
                                 r(qk_sb[:, 0:256]), start=True, stop=True)
                # v natural [t, h]: psum [128, 128]: cols 0:64 t-chunk0, 64:128 t-chunk1
                v_ps = vp.tile([128, 128], f32)
                nc.tensor.matmul(v_ps[:, 0:64], r(xt[:, 0:128]), r(wv_sb[:, 0:64]),
                                 start=True, stop=False)
                nc.tensor.matmul(v_ps[:, 0:64], r(xt[:, 256:384]), r(wv_sb[:, 64:128]),
                                 start=False, stop=True)
                nc.tensor.matmul(v_ps[:, 64:128], r(xt[:, 128:256]), r(wv_sb[:, 0:64]),
                                 start=True, stop=False)
                nc.tensor.matmul(v_ps[:, 64:128], r(xt[:, 384:512]), r(wv_sb[:, 64:128]),
                                 start=False, stop=True)
                # v_aug [tk-chunk part, 2*(64+1)]: cols 0:64 v(t0), 64 ones, 65:129 v(t1), 129 ones
                v_sb = vs.tile([128, 130], f32r)
                nc.vector.tensor_copy(
                    v_sb[:].rearrange("p (g d) -> p g d", d=65)[:, :, 0:64],
                    v_ps[:].rearrange("p (g d) -> p g d", d=64),
                )
                nc.vector.tensor_copy(
                    v_sb[:].rearrange("p (g d) -> p g d", d=65)[:, :, 64:65],
                    ones_sb[:].rearrange("p (g d) -> p g d", d=1),
                )

                # causal mask: even items additive on DVE pre-exp; odd items
                # multiplicative 0/1 on GpSimd post-exp (engine balance)
                expT_sb = exps.tile([128, 512], f32r)
                if b % 2 == 0:
                    nc.vector.tensor_add(wei_ps[:], wei_ps[:], mask_sb[:])
                    nc.scalar.activation(expT_sb[:], wei_ps[:], AF.Exp)
                else:
                    nc.scalar.activation(expT_sb[:], wei_ps[:], AF.Exp)
                    nc.gpsimd.tensor_mul(expT_sb[:], expT_sb[:], mmask_sb[:])

                # out_aug^T [65, 256]: rows 0:64 = (expT^T v)^T, row 64 = denom
                out_ps = outp.tile([HS + 1, 256], f32)
                nc.tensor.matmul(out_ps[:], r(v_sb[:, 0:65]), r(expT_sb[:, 0:256]),
                                 start=True, stop=False)
                nc.tensor.matmul(out_ps[:], r(v_sb[:, 65:130]), r(expT_sb[:, 256:512]),
                                 start=False, stop=True)
                out_sb = outs.tile([HS + 1, 256], f32)
                if b % 2 == 1 or (b // 2) % 2 == 1:
                    nc.scalar.copy(out_sb[:], out_ps[:])
                else:
                    nc.vector.tensor_copy(out_sb[:], out_ps[:])
                nc.sync.dma_start(out_d[b], out_sb[:])

    nc.compile()
    return nc


def kernel(x, Wq, Wk, Wv):
    global LAST_RESULT, _COMPILED
    from concourse import bass_utils

    if _COMPILED is None:
        _COMPILED = _build_program()
    nc = _COMPILED

    scale = np.float32(C ** -0.5)
    wqk = np.concatenate([Wq.astype(np.float32) * scale, Wk.astype(np.float32)],
                         axis=1)  # [C, 128]
    xT = np.ascontiguousarray(x.astype(np.float32).transpose(0, 2, 1))  # [B, C, T]

    # additive causal mask in weiT layout [tk(2 chunks of 128), tq 256]
    tq = np.arange(T)
    mask = np.zeros((128, 512), dtype=np.float32)
    for chunk in range(2):
        tk = np.arange(128) + chunk * 128
        blk = np.where(tq[None, :] >= tk[:, None], 0.0, -1e5).astype(np.float32)
        mask[:, chunk * 256:(chunk + 1) * 256] = blk

    in_maps = []
    for ci in range(NCORES):
        in_maps.append({
            "xT": xT[ci * BPC:(ci + 1) * BPC],
            "Wqk": wqk,
            "Wv": np.ascontiguousarray(Wv.astype(np.float32)),
            "maskT": mask,
            "ones": np.ones((128, 2), dtype=np.float32),
            "maskM": (mask == 0.0).astype(np.float32),
        })

    res = bass_utils.run_bass_kernel_spmd(nc, in_maps, core_ids=list(range(NCORES)))
    LAST_RESULT = res

    out_aug = np.concatenate([r["out_aug"] for r in res.results], axis=0)  # [B,65,T]
    out = out_aug[:, :HS, :] / out_aug[:, HS:HS + 1, :]
    return np.ascontiguousarray(out.transpose(0, 2, 1)).astype(np.float32)
